# revision 1
# baseline (speedup 1.0000x reference)
"""Trainium2 Bass kernel for a dense multi-head attention layer.

Problem: B=4, S=2048, D=1024, H=16, DH=64 attention (QKV projections +
softmax(QK^T/sqrt(DH))V), fp32 reference, attention_mask all-ones, zero
biases.

Sharding (8 NeuronCores): core c handles batch b=c//2 and head-half
hh=c%2 (8 of 16 heads).  Per-core work is perfectly balanced with no
collectives: each core projects its 8 heads' Q/K/V over the full
sequence of its batch and runs attention for those heads.

Per-core device algorithm (all matmuls bf16 in / fp32 PSUM accumulate):
  - Q^T, K^T computed in [outcol, token] layout (lhsT = W, rhs = X^T).
  - V computed in [token, outcol] layout (lhsT = X^T tile, rhs = W),
    stored per (t_tile, head) with a constant ones column appended.
  - scores^T[t, f] per head via lhsT=K^T tile (contraction dh=64); two
    heads of an outcol-tile are row-packed into array rows 0-63/64-127
    so both stream concurrently.
  - exp via ScalarE activation (scale=1/8 fused) straight out of PSUM
    in 3-bank supertiles -> bf16 expT in SBUF.  No max-subtraction
    (scores are O(1); softmax is shift-invariant).
  - PV: ctx^T[dh,f] = sum_t V[t,dh]*expT[t,f] with lhsT=[V|1] (M=65);
    row 64 accumulates the softmax denominator for free.
  - normalize: reciprocal of the denominator row, DRAM-bounce broadcast
    across 64 partitions, DVE multiply; DMA out as ctx^T [512, 2048].
Host reassembles: out[b, :, hh*512:(hh+1)*512] = core_out.T
"""

import numpy as np
import ml_dtypes

B, S, D = 4, 2048, 1024
H, DH = 16, 64
NCORES = 8
HL = 8            # local heads per core
OC = HL * DH      # 512 local output columns
P = 128
NDC = D // P      # 8 contraction chunks for projections
FB = 512          # f-block (query) width
NFB = S // FB     # 4
NTT = S // P      # 16 key tiles
SCALE = 1.0 / np.sqrt(DH)
# t-tile groups per ScalarE activation (3 PSUM banks each; last is 1)
GROUPS = [(0, 3), (3, 3), (6, 3), (9, 3), (12, 3), (15, 1)]

_CACHE = {}


def _build_nc():
    import concourse.bass as bass
    import concourse.tile as tile
    from concourse import bacc, mybir
    from concourse.bass import ts, ds

    bf16 = mybir.dt.bfloat16
    f32 = mybir.dt.float32
    Exp = mybir.ActivationFunctionType.Exp

    nc = bacc.Bacc("TRN2", target_bir_lowering=False, debug=False)

    xfT_d = nc.dram_tensor("xfT", [D, S], bf16, kind="ExternalInput")
    xtT_d = nc.dram_tensor("xtT", [D, S], bf16, kind="ExternalInput")
    wq_d = nc.dram_tensor("wq", [D, OC], bf16, kind="ExternalInput")
    wk_d = nc.dram_tensor("wk", [D, OC], bf16, kind="ExternalInput")
    wv_d = nc.dram_tensor("wv", [D, OC], bf16, kind="ExternalInput")
    out_d = nc.dram_tensor("out", [OC, S], f32, kind="ExternalOutput")

    with tile.TileContext(nc) as tc:
        with (
            tc.tile_pool(name="persist", bufs=1) as pp,
            tc.tile_pool(name="ps_sc", bufs=2, space="PSUM") as ps_sc,
            tc.tile_pool(name="ps_b1", bufs=2, space="PSUM") as ps_b1,
        ):
            qT = pp.tile([P, 4, S], bf16, tag="qT")
            kT = pp.tile([P, 4, S], bf16, tag="kT")
            v = pp.tile([P, NTT, HL, DH + 1], bf16, tag="v")
            nc.vector.memset(v[:, :, :, DH], 1.0)

            # ---------- phase 1: projections ----------
            with tc.tile_pool(name="proj_in", bufs=1) as pin:
                xfT = pin.tile([P, NDC, S], bf16, tag="xfT")
                xtT = pin.tile([P, NDC, S], bf16, tag="xtT")
                wq = pin.tile([P, NDC, OC], bf16, tag="wq")
                wk = pin.tile([P, NDC, OC], bf16, tag="wk")
                wv = pin.tile([P, NDC, OC], bf16, tag="wv")
                for sb_t, dr in ((xfT, xfT_d), (xtT, xtT_d), (wq, wq_d),
                                 (wk, wk_d), (wv, wv_d)):
                    nc.sync.dma_start(
                        out=sb_t[:],
                        in_=dr.ap().rearrange("(c p) n -> p c n", p=P),
                    )

                # Q^T and K^T in [outcol, token] layout
                for w_sb, x_sb, dst in ((wq, xfT, qT), (wk, xtT, kT)):
                    for ot in range(4):
                        for tch in range(4):
                            psq = ps_b1.tile([P, FB], f32, tag="b1")
                            for dc in range(NDC):
                                nc.tensor.matmul(
                                    psq[:],
                                    w_sb[:, dc, ts(ot, P)],
                                    x_sb[:, dc, ts(tch, FB)],
                                    start=(dc == 0),
                                    stop=(dc == NDC - 1),
                                )
                            nc.vector.tensor_copy(
                                dst[:, ot, ts(tch, FB)], psq[:]
                            )

                # V in [token, outcol] layout, scattered per (t_tile, head)
                for tt in range(NTT):
                    psv = ps_b1.tile([P, FB], f32, tag="b1")
                    for dc in range(NDC):
                        nc.tensor.matmul(
                            psv[:],
                            xtT[:, dc, ts(tt, P)],
                            wv[:, dc, :],
                            start=(dc == 0),
                            stop=(dc == NDC - 1),
                        )
                    nc.vector.tensor_copy(
                        v[:, tt, :, 0:DH],
                        psv[:].rearrange("p (h d) -> p h d", h=HL),
                    )

            # ---------- phase 2: attention ----------
            with (
                tc.tile_pool(name="expt", bufs=2) as ep,
                tc.tile_pool(name="small", bufs=4) as sp,
                tc.tile_pool(name="dscr", bufs=4, space="DRAM") as dp,
            ):
                for j in range(4):          # outcol tile = head pair (2j, 2j+1)
                    for fb in range(NFB):
                        eA = ep.tile([P, NTT, FB], bf16, tag="eA")
                        eB = ep.tile([P, NTT, FB], bf16, tag="eB")
                        for g0, glen in GROUPS:
                            scA = ps_sc.tile([P, 3, FB], f32, tag="sc")
                            scB = ps_sc.tile([P, 3, FB], f32, tag="sc")
                            for t in range(glen):
                                tt = g0 + t
                                nc.tensor.matmul(
                                    scA[:, t, :],
                                    kT[0:64, j, ts(tt, P)],
                                    qT[0:64, j, ts(fb, FB)],
                                    start=True, stop=True,
                                    tile_position=(0, 0),
                                )
                                nc.tensor.matmul(
                                    scB[:, t, :],
                                    kT[64:128, j, ts(tt, P)],
                                    qT[64:128, j, ts(fb, FB)],
                                    start=True, stop=True,
                                    tile_position=(64, 0),
                                )
                            nc.scalar.activation(
                                eA[:, g0:g0 + glen, :], scA[:, 0:glen, :],
                                Exp, scale=float(SCALE),
                            )
                            nc.scalar.activation(
                                eB[:, g0:g0 + glen, :], scB[:, 0:glen, :],
                                Exp, scale=float(SCALE),
                            )
                        for hl, e in ((2 * j, eA), (2 * j + 1, eB)):
                            cps = ps_b1.tile([DH + 1, FB], f32, tag="b1")
                            for tt in range(NTT):
                                nc.tensor.matmul(
                                    cps[:],
                                    v[:, tt, hl, :],
                                    e[:, tt, :],
                                    start=(tt == 0),
                                    stop=(tt == NTT - 1),
                                )
                            recip = sp.tile([1, FB], f32, tag="recip")
                            nc.vector.reciprocal(recip[:], cps[DH:DH + 1, :])
                            dscr = dp.tile([1, FB], f32, tag="d")
                            nc.sync.dma_start(out=dscr[:], in_=recip[:])
                            rbc = sp.tile([DH, FB], f32, tag="rbc")
                            nc.gpsimd.dma_start(
                                out=rbc[:],
                                in_=bass.AP(
                                    tensor=dscr.tensor,
                                    offset=dscr.offset,
                                    ap=[[0, DH], [1, FB]],
                                ),
                            )
                            outst = sp.tile([DH, FB], f32, tag="outst")
                            nc.vector.tensor_mul(
                                outst[:], cps[0:DH, :], rbc[:]
                            )
                            nc.sync.dma_start(
                                out=out_d.ap()[ds(hl * DH, DH), ts(fb, FB)],
                                in_=outst[:],
                            )
    nc.compile()
    return nc


def _get_nc():
    if "nc" not in _CACHE:
        _CACHE["nc"] = _build_nc()
    return _CACHE["nc"]


def _numpy_reference(x_from, x_to, attention_mask, wq, bq, wk, bk, wv, bv):
    """General fallback (used only if mask/biases are not the expected
    all-ones / zeros of this problem instance)."""
    b, fs, _ = x_from.shape
    ts_ = x_to.shape[1]
    q = (x_from @ wq + bq).reshape(b, fs, H, DH).transpose(0, 2, 1, 3)
    k = (x_to @ wk + bk).reshape(b, ts_, H, DH).transpose(0, 2, 1, 3)
    v = (x_to @ wv + bv).reshape(b, ts_, H, DH).transpose(0, 2, 1, 3)
    scores = np.einsum("bhfd,bhtd->bhft", q, k) * (1.0 / np.sqrt(DH))
    adder = (1.0 - attention_mask[:, None, :, :].astype(np.float32)) * -10000.0
    scores = scores + adder
    scores -= scores.max(axis=-1, keepdims=True)
    e = np.exp(scores)
    probs = e / e.sum(axis=-1, keepdims=True)
    ctx = np.einsum("bhft,bhtd->bhfd", probs, v)
    return ctx.transpose(0, 2, 1, 3).reshape(b, fs, H * DH).astype(np.float32)


def _make_in_maps(x_from, x_to, wq, wk, wv):
    bf = ml_dtypes.bfloat16
    xfT = [np.ascontiguousarray(x_from[b].T).astype(bf) for b in range(B)]
    xtT = [np.ascontiguousarray(x_to[b].T).astype(bf) for b in range(B)]
    wq_h = [np.ascontiguousarray(wq[:, hh * OC:(hh + 1) * OC]).astype(bf)
            for hh in range(2)]
    wk_h = [np.ascontiguousarray(wk[:, hh * OC:(hh + 1) * OC]).astype(bf)
            for hh in range(2)]
    wv_h = [np.ascontiguousarray(wv[:, hh * OC:(hh + 1) * OC]).astype(bf)
            for hh in range(2)]
    in_maps = []
    for c in range(NCORES):
        b, hh = c // 2, c % 2
        in_maps.append({
            "xfT": xfT[b], "xtT": xtT[b],
            "wq": wq_h[hh], "wk": wk_h[hh], "wv": wv_h[hh],
        })
    return in_maps


def _assemble(results):
    out = np.empty((B, S, H * DH), np.float32)
    for c in range(NCORES):
        b, hh = c // 2, c % 2
        out[b, :, hh * OC:(hh + 1) * OC] = results[c]["out"].T
    return out


def _run(inputs, **spmd_kwargs):
    x_from = np.asarray(inputs["x_from"], dtype=np.float32)
    x_to = np.asarray(inputs["x_to"], dtype=np.float32)
    mask = np.asarray(inputs["attention_mask"])
    wq = np.asarray(inputs["wq"], dtype=np.float32)
    wk = np.asarray(inputs["wk"], dtype=np.float32)
    wv = np.asarray(inputs["wv"], dtype=np.float32)
    bq = np.asarray(inputs["bq"], dtype=np.float32)
    bk = np.asarray(inputs["bk"], dtype=np.float32)
    bv = np.asarray(inputs["bv"], dtype=np.float32)

    if (mask != 1).any() or bq.any() or bk.any() or bv.any():
        return _numpy_reference(x_from, x_to, mask, wq, bq, wk, bk, wv, bv), None

    from concourse.bass_utils import run_bass_kernel_spmd

    nc = _get_nc()
    in_maps = _make_in_maps(x_from, x_to, wq, wk, wv)
    res = run_bass_kernel_spmd(nc, in_maps, list(range(NCORES)), **spmd_kwargs)
    return _assemble(res.results), res


def kernel(**inputs) -> np.ndarray:
    out, _ = _run(inputs)
    return out


def kernel_traced(**inputs):
    """Like kernel() but also returns the BassKernelResults (with
    exec_time_ns / profile when NTFF tracing is available)."""
    return _run(inputs, trace=True)
